# revision 61
# baseline (speedup 1.0000x reference)
"""Multi-head attention (B=2, N=2048, EMB=1024, H=16, hd=64) on 8 TRN2 NeuronCores.

Sharding: tensor-parallel over heads. Each core owns 2 heads: it gets the
W_qkv columns (k|q|v sections) and W_out rows for those heads, computes
QKV projection + attention + its partial output projection, and the host
sums the 8 partials (the "all-reduce") and adds b_out.

Device kernel layout (per core), all matmuls bf16 with fp32 PSUM accumulation:
  - x is pre-transposed on host to xT [EMB, TOK] so the embedding dim lands on
    SBUF partitions (matmul contraction dim).
  - QKV^T is produced in [dims, tokens] layout: lhsT = W chunk, rhs = xT chunk.
    K/Q sections stay transposed ([hd, tok]) for the scores matmul; the V
    section is produced directly in [tok, hd] tiles (vaug) via plain matmuls
    with lhsT = xT chunk.
  - vaug per (b, i, h): head 0 = [v dims @ cols 0..63 | ones @ col 64],
    head 1 = [ones @ col 63 | v dims @ cols 64..127].  attn@V therefore puts
    head-1 dims straight onto PSUM partitions 64..127 (and its softmax
    denominator on row 63), so the normalized output lands on the partitions
    the out-projection needs without any partition-shift DMA.
  - scores^T chunks [k_tok 128, q 512] per head via row-packed (K=64) matmuls
    (the two heads run concurrently in disjoint PE row groups).  Entering or
    leaving row-tiled mode costs ~100ns of PE pipeline flush, so score groups
    for two consecutive k-chunks are emitted back-to-back (the penalty is
    paid once per pair).  exp on ScalarE straight out of PSUM (scale=1/8
    folded in, no max subtraction needed: scores ~ N(0,1)), bf16 expT.
  - attn@V: lhsT = vaug [k_tok 128, 128], accumulated over 16 k chunks, with
    a global 4-chunk lag behind the exp stream.  The chunk stream is a single
    software pipeline across ALL 8 (batch, q-quarter) units, so the PE never
    drains at unit boundaries.
  - normalize: psa evicted bf16 (h0 rows 0..64, h1 rows 63..127); den rows
    DMA-packed to one [2, 512] tile; one Ln + one Exp (scale=-1) per unit on
    ScalarE (same table set as the bulk exp), emitted as two separate filler
    quanta so the bulk-exp stream sees two small bumps, not one 1.4us stall;
    partition-broadcast by a single zero-padded K=128 selector-matmul (plain
    full-array op, no tile-mode switch); multiplies on VectorE into A_norm.
  - out projection: lhsT = A_norm chunk [128, 128], rhs = W_out shard
    [128, 512], PSUM -> SBUF bf16 -> DRAM partial [TOK, EMB] bf16.
"""

import os
from collections import deque

import numpy as np
import ml_dtypes

B = 2
N = 2048
EMB = 1024
TOK = B * N  # 4096
HD = 64
H_PER_CORE = 2
DIMS = 3 * H_PER_CORE * HD  # 384 qkv cols per core
ATT_LOCAL = H_PER_CORE * HD  # 128
P = 128
EC = EMB // P  # 8 embedding chunks
TCQ = TOK // 512  # 8 token chunks for the qkv projection
KCH = N // P  # 16 key chunks per batch
QQ = N // 512  # 4 query quarters per batch
NUNITS = B * QQ  # 8
GCH = NUNITS * KCH  # 128 global chunks
SCALE = HD ** -0.5

_CACHE = {}
LAST = {}


def _patch_act_tables():
    # Route Exp to natural_log_exp_and_others so the per-unit Ln calls and
    # the bulk Exp calls share one table set (no ACT_TABLE_LOAD thrash).
    # Entries keep their order, so act_func_set_id indices stay valid.
    import concourse.bacc as bacc_mod
    from concourse import mybir

    if getattr(bacc_mod, "_act_tables_patched", False):
        return
    orig = bacc_mod.get_activation_tables

    def patched(arch):
        t = orig(arch)
        E = mybir.ActivationFunctionType.Exp
        if "natural_log_exp_and_others" in t:
            for name, fns in t.items():
                if name != "natural_log_exp_and_others" and E in fns:
                    t[name] = fns - {E}
        return t

    bacc_mod.get_activation_tables = patched
    bacc_mod._act_tables_patched = True


def _build_graph():
    from concourse import bacc, mybir
    import concourse.tile as tile

    _patch_act_tables()

    nc = bacc.Bacc(
        "TRN2", target_bir_lowering=False, debug=False, num_devices=1
    )
    dt = mybir.dt
    xT = nc.dram_tensor("xT", [EMB, TOK], dt.bfloat16, kind="ExternalInput")
    wqkv = nc.dram_tensor("wqkv", [EMB, DIMS], dt.bfloat16, kind="ExternalInput")
    bqkv = nc.dram_tensor("bqkv", [DIMS], dt.float32, kind="ExternalInput")
    wout = nc.dram_tensor("wout", [ATT_LOCAL, EMB], dt.bfloat16, kind="ExternalInput")
    out = nc.dram_tensor("out", [TOK, EMB], dt.bfloat16, kind="ExternalOutput")

    with tile.TileContext(nc) as tc:
        _emit(tc, nc, xT, wqkv, bqkv, wout, out)
    nc.compile()
    return nc


def _emit(tc, nc, xT, wqkv, bqkv, wout, out):
    from contextlib import ExitStack
    import concourse.bass as bass
    from concourse import mybir
    from concourse.masks import make_identity

    dt = mybir.dt
    f32, bf16 = dt.float32, dt.bfloat16
    Exp = mybir.ActivationFunctionType.Exp
    Ln = mybir.ActivationFunctionType.Ln

    with ExitStack() as ctx:
        consts = ctx.enter_context(tc.tile_pool(name="consts", bufs=1))
        xt_pool = ctx.enter_context(tc.tile_pool(name="xt", bufs=3))
        persist = ctx.enter_context(tc.tile_pool(name="persist", bufs=1))
        expp = ctx.enter_context(tc.tile_pool(name="expp", bufs=8))
        small = ctx.enter_context(tc.tile_pool(name="small", bufs=8))
        outst = ctx.enter_context(tc.tile_pool(name="outst", bufs=4))
        ps_scores = ctx.enter_context(
            tc.tile_pool(name="ps_scores", bufs=2, space="PSUM")
        )
        ps_att = ctx.enter_context(tc.tile_pool(name="ps_att", bufs=2, space="PSUM"))
        ps_small = ctx.enter_context(
            tc.tile_pool(name="ps_small", bufs=2, space="PSUM")
        )

        # ---- early DMAs: first xT chunk + weights, so the QKV stream can
        # start the moment the warmup matmuls finish ----
        xts = {}

        def dma_xt(t, engs=None):
            # 8 whole-chunk DMAs: finer splits would halve per-queue transfer
            # time but each extra dma_start costs ~0.6us on the issuing
            # engine's queue, which becomes the bottleneck (measured: 235
            # issues put Sync at 63% busy and cost 31us of span).  At startup
            # the issues alternate Sync/Scalar so the 8-deep issue wall the
            # first QKV chain waits behind is halved.
            if engs is None:
                engs = (nc.sync,)
            xt = xt_pool.tile([P, EC, 512], bf16, tag="xt")
            for e in range(EC):
                engs[e % len(engs)].dma_start(
                    out=xt[:, e, :], in_=xT[e * P : (e + 1) * P, bass.ts(t, 512)]
                )
            xts[t] = xt

        dma_xt(0, engs=(nc.sync, nc.scalar))
        w_sb = consts.tile([P, EC, DIMS], bf16, tag="w_sb")
        for e in range(EC):
            eng = (nc.scalar, nc.sync)[e % 2]
            eng.dma_start(out=w_sb[:, e, :], in_=wqkv[e * P : (e + 1) * P, :])
        bias_sb = consts.tile([P, 3], f32, tag="bias_sb")
        nc.sync.dma_start(out=bias_sb, in_=bqkv[:].rearrange("(c p) -> p c", p=P))
        dma_xt(1)
        wout_sb = consts.tile([P, EMB], bf16, tag="wout_sb")
        nc.sync.dma_start(out=wout_sb, in_=wout[:, :])

        # warm up the exp table set as early as possible (one-time ~2.7us)
        warm = consts.tile([1, 8], f32, tag="warm")
        nc.vector.memset(warm, 1.0)
        nc.scalar.activation(out=warm, in_=warm, func=Ln, scale=1.0)
        nc.scalar.activation(out=warm, in_=warm, func=Exp, scale=1.0)

        ident = consts.tile([P, P], bf16, tag="ident")
        make_identity(nc, ident)
        # selector for the 1/den partition-broadcast: rows 0..63 get rc2[0]
        # (head 0), rows 64..127 get rc2[1] (head 1)
        # Selector for the 1/den partition-broadcast, zero-padded to K=128 so
        # the broadcast matmul is a plain full-array op (no row-tiled-mode
        # switch penalty).  Row 1 is staged on partition 0 and DMA'd into
        # place (engine ops need 32-aligned partition bases).  rc2full rows
        # 2..127 are zeroed once so the padded contraction adds exact zeros.
        sel = consts.tile([P, P], bf16, tag="sel")
        nc.vector.memset(sel, 0.0)
        nc.vector.memset(sel[0:1, 0:HD], 1.0)
        selrow = consts.tile([1, P], bf16, tag="selrow")
        nc.vector.memset(selrow, 0.0)
        nc.vector.memset(selrow[0:1, HD:P], 1.0)
        nc.sync.dma_start(out=sel[1:2, :], in_=selrow)
        rc2full = consts.tile([P, 512], bf16, tag="rc2full")
        nc.vector.memset(rc2full, 0.0)
        # tail-only selector: broadcast from rc2full rows 64 (h0 den) and
        # 32 (h1 den), both 32-aligned so they can be written in-place
        selT = consts.tile([P, P], bf16, tag="selT")
        nc.vector.memset(selT, 0.0)
        nc.vector.memset(selT[HD : HD + 1, 0:HD], 1.0)
        nc.vector.memset(selT[32:33, HD:P], 1.0)

        # [tok 128, b, chunk, h, 128]: h0 = [dims | ones@64 | junk],
        # h1 = [junk | ones@63 | dims@64..127].  Junk columns only feed
        # output rows that are never read.
        vaug = persist.tile([P, B, KCH, H_PER_CORE, P], bf16, tag="vaug")
        nc.vector.memset(vaug[:, :, :, :, :], 0.0)
        nc.vector.memset(vaug[:, :, :, 0, HD : HD + 1], 1.0)
        # h1's denominator ones-column sits at col 32 (not 63) so its PSUM
        # row lands on a 32-aligned partition: the tail can then run Ln/Exp
        # directly on the evicted tile without a partition-packing DMA
        nc.vector.memset(vaug[:, :, :, 1, 32:33], 1.0)

        k_sb = persist.tile([P, TOK], bf16, tag="k_sb")
        q_sb = persist.tile([P, TOK], bf16, tag="q_sb")
        anorm = persist.tile([P, TOK], bf16, tag="anorm")

        # ---- PE warmup: identity matmuls keep the PE busy through the HAM
        # activity window while the first DMAs land, so the QKV stream runs
        # at the warm 2.4 GHz clock from its first matmul ----
        wm_ps = ps_small.tile([P, 512], f32, tag="ps_small", name="wm_ps")
        for _ in range(84):
            nc.tensor.matmul(
                wm_ps[:, 0:128], lhsT=ident, rhs=ident, start=True, stop=True
            )

        qkv_dst = (k_sb, q_sb)

        def qkv_mms(t, d, e0, e1, ps):
            for e in range(e0, e1):
                nc.tensor.matmul(
                    ps,
                    lhsT=w_sb[:, e, d * P : (d + 1) * P],
                    rhs=xts[t][:, e, :],
                    start=(e == 0),
                    stop=(e == EC - 1),
                )

        def qkv_bias(t, d, ps):
            nc.vector.tensor_scalar_add(
                out=qkv_dst[d][:, bass.ts(t, 512)],
                in0=ps,
                scalar1=bias_sb[:, d : d + 1],
            )

        def v_mms(t, tci, ps):
            # V[tok, dim] for tokens t*512+tci*128 .. +128, accumulated into
            # column range tci*128..+128 of a shared [128, 512] PSUM bank
            # (one eviction copy per 4 sub-chunks).
            for e in range(EC):
                nc.tensor.matmul(
                    ps[:, tci * P : (tci + 1) * P],
                    lhsT=xts[t][:, e, tci * P : (tci + 1) * P],
                    rhs=w_sb[:, e, 2 * P : 3 * P],
                    start=(e == 0),
                    stop=(e == EC - 1),
                )

        def v_copy(t, ps):
            # ps cols [tci*128+h*64 .. +64] = head h dims for token sub-chunk
            # tci.  h0 -> vaug cols 0..64, h1 -> vaug cols 64..128.
            gtok = t * 512
            b, i0 = gtok // N, (gtok % N) // P
            src = ps.rearrange("p (c h d) -> p c h d", c=4, h=2)
            nc.vector.tensor_copy(
                out=vaug[:, b, i0 : i0 + 4, 0, 0:HD], in_=src[:, :, 0, :]
            )
            nc.vector.tensor_copy(
                out=vaug[:, b, i0 : i0 + 4, 1, HD:P], in_=src[:, :, 1, :]
            )

        def v_copy2(t, tci0, ps):
            # same as v_copy but for the tci pair [tci0, tci0+1]
            gtok = t * 512
            b, i0 = gtok // N, (gtok % N) // P
            src = ps[:, tci0 * P : (tci0 + 2) * P].rearrange(
                "p (c h d) -> p c h d", c=2, h=2
            )
            nc.vector.tensor_copy(
                out=vaug[:, b, i0 + tci0 : i0 + tci0 + 2, 0, 0:HD],
                in_=src[:, :, 0, :],
            )
            nc.vector.tensor_copy(
                out=vaug[:, b, i0 + tci0 : i0 + tci0 + 2, 1, HD:P],
                in_=src[:, :, 1, :],
            )

        # ---- filler task queue: (taskid, pe_cost_ns, fn) ----
        fillerq = deque()
        task_deadline = {}

        def enq_task(taskid, deadline, quanta):
            task_deadline[taskid] = deadline
            for cost, fn in quanta:
                fillerq.append((taskid, cost, fn))

        def run_quantum():
            _, _, fn = fillerq.popleft()
            fn()

        def pump(g, budget):
            # run everything whose deadline is upon us, then fill the budget
            while fillerq and task_deadline[fillerq[0][0]] <= g + 1:
                run_quantum()
            while fillerq and budget > 0:
                tid, cost, _ = fillerq[0]
                run_quantum()
                budget -= cost

        # Each quantum fully contains the lifetime of any ps_small tile it
        # allocates (alloc + all writes + final reader), so the pool's
        # round-robin reuse can never interleave with a half-written bank.
        def make_qkv_task(t):
            quanta = []
            for d in range(2):

                def q(d=d):
                    ps = ps_small.tile(
                        [P, 512], f32, tag="ps_small", name=f"qkv{t}_{d}"
                    )
                    qkv_mms(t, d, 0, EC, ps)
                    qkv_bias(t, d, ps)

                quanta.append((1760, q))
            return quanta

        def make_v_task(t):
            # two ~0.9us quanta (tci pairs) instead of one 1.8us block, so a
            # popped v quantum delays the next scores batch half as long.
            # Each quantum owns its PSUM bank for its full lifetime.
            def q(tci0):
                ps = ps_small.tile(
                    [P, 512], f32, tag="ps_small", name=f"v{t}_{tci0}"
                )
                for tci in (tci0, tci0 + 1):
                    v_mms(t, tci, ps)
                v_copy2(t, tci0, ps)

            return [(900, lambda: q(0)), (900, lambda: q(2))]

        # t=0 runs inline before the stream; t=1..7 go through the queue.
        # Deadlines (global chunk index): batch-0 k-side needs qkv(t) by
        # chunk 4t; batch-1 needs qkv(4+j) by chunk 64+4j.  The xT DMA for
        # t+1 is enqueued ahead of qkv(t)'s matmuls so the transfer overlaps
        # a full task's worth of compute.
        def qkv_ddl(t):
            return 4 * t if t < 4 else 64 + 4 * (t - 4)

        # v0 is only needed by the first attn@V (chunk 4): run it as the
        # first filler quanta instead of ahead of the first scores pair
        enq_task("v0", 2, make_v_task(0))
        for t in range(1, TCQ):
            if t + 1 < TCQ:
                enq_task(
                    f"dma{t + 1}",
                    max(qkv_ddl(t + 1) - 4, 0),
                    [(0, lambda t2=t + 1: dma_xt(t2))],
                )
            enq_task(f"qkv{t}", qkv_ddl(t), make_qkv_task(t))
            enq_task(f"v{t}", qkv_ddl(t) + 2, make_v_task(t))

        # ---- t=0 inline (the stream's first chunks need it) ----
        # The K and Q accumulation chains interleave per e-chunk: the first
        # QKV is gated on the staggered xt0 DMA arrivals, so this way both
        # chains finish ~one chain-length sooner after the last arrival.
        ps0 = ps_small.tile([P, 512], f32, tag="ps_small", name="qkv0_0")
        ps1 = ps_small.tile([P, 512], f32, tag="ps_small", name="qkv0_1")
        for e in range(EC):
            for d, ps in ((0, ps0), (1, ps1)):
                nc.tensor.matmul(
                    ps,
                    lhsT=w_sb[:, e, d * P : (d + 1) * P],
                    rhs=xts[0][:, e, :],
                    start=(e == 0),
                    stop=(e == EC - 1),
                )
        qkv_bias(0, 0, ps0)
        qkv_bias(0, 1, ps1)

        # ---- per-unit finish machinery ----
        fin = {}

        def drain_unit(u, ps_a):
            # evict accumulators to SBUF (bf16): h0 rows 0..64 (den at 64),
            # h1 rows 63..127 (den at 63)
            psa0 = small.tile([P, 512], bf16, tag="psa_sb")
            nc.vector.tensor_copy(out=psa0[0 : HD + 1, :], in_=ps_a[0][0 : HD + 1, :])
            psa1 = small.tile([P, 512], bf16, tag="psa_sb")
            # non-zero partition bases are limited to 32-partition accesses,
            # and cost scales with the free dim only, so copy the full tile
            # (rows 0..62 are junk, den at 63, dims at 64..127).  For the
            # last unit the copy runs on ScalarE (idle in the tail) so both
            # evictions overlap instead of serializing on VectorE.
            if u == NUNITS - 1:
                nc.scalar.copy(out=psa1, in_=ps_a[1][:, :])
            else:
                nc.vector.tensor_copy(out=psa1, in_=ps_a[1][:, :])
            den2 = small.tile([2, 512], bf16, tag="den2")
            nc.sync.dma_start(out=den2[0:1, :], in_=psa0[HD : HD + 1, :])
            nc.sync.dma_start(out=den2[1:2, :], in_=psa1[32:33, :])
            fin[u] = (psa0, psa1, den2)

        # 1/den = exp(-ln(den)) on ScalarE, one [2, 512] call per func;
        # partition-broadcast via a single K=128 zero-padded selector
        # matmul; then normalize multiplies on VectorE.  Split into two
        # quanta so the ScalarE insertions are two ~0.7us bumps instead of
        # one 1.4us block that stalls the bulk-exp stream.
        def finish_a(u):
            _, _, den2 = fin[u]
            tln = small.tile([2, 512], f32, tag="tln")
            nc.scalar.activation(out=tln, in_=den2, func=Ln)
            fin[u] = fin[u][:2] + (tln,)

        def finish_b(u):
            psa0, psa1, tln = fin.pop(u)
            b, qq = divmod(u, QQ)
            qbase = b * N + qq * 512
            nc.scalar.activation(out=rc2full[0:2, :], in_=tln, func=Exp, scale=-1.0)
            rrep = ps_small.tile([P, 512], f32, tag="ps_small", name=f"rrep{u}")
            nc.tensor.matmul(rrep, lhsT=sel, rhs=rc2full, start=True, stop=True)
            nc.vector.tensor_mul(
                out=anorm[0:HD, qbase : qbase + 512],
                in0=psa0[0:HD, :],
                in1=rrep[0:HD, :],
            )
            # >32-partition accesses must start at partition 0, so the h1
            # multiply goes in two 32-partition pieces
            for p0 in (HD, HD + 32):
                nc.vector.tensor_mul(
                    out=anorm[p0 : p0 + 32, qbase : qbase + 512],
                    in0=psa1[p0 : p0 + 32, :],
                    in1=rrep[p0 : p0 + 32, :],
                )

        def finish_unit(u):
            finish_a(u)
            finish_b(u)

        def outproj_chunk(u, tci):
            b, qq = divmod(u, QQ)
            tok0 = b * N + qq * 512 + tci * P
            ob = outst.tile([P, EMB], bf16, tag="outst")
            for e2 in range(2):
                ps = ps_small.tile([P, 512], f32, tag="ps_small")
                nc.tensor.matmul(
                    ps,
                    lhsT=anorm[:, tok0 : tok0 + P],
                    rhs=wout_sb[:, e2 * 512 : (e2 + 1) * 512],
                    start=True,
                    stop=True,
                )
                nc.vector.tensor_copy(out=ob[:, e2 * 512 : (e2 + 1) * 512], in_=ps)
            nc.sync.dma_start(out=out[tok0 : tok0 + P, :], in_=ob)

        # ---- the global chunk stream ----
        # One software pipeline over all 128 (unit, k-chunk) pairs: attn@V
        # lags the exp stream by 2 chunks and flows straight across unit
        # boundaries, so the PE never waits for an exp at a boundary.
        ps_a = {}
        lag = []

        def attnv(u, i, ex):
            b = u // QQ
            for h in range(H_PER_CORE):
                nc.tensor.matmul(
                    ps_a[u][h],
                    lhsT=vaug[:, b, i, h, :],
                    rhs=ex[:, h * 512 : (h + 1) * 512],
                    start=(i == 0),
                    stop=(i == KCH - 1),
                )
            if i == KCH - 1:
                drain_unit(u, ps_a.pop(u))

        # Chunks are processed in PAIRS: the two row-packed score groups are
        # emitted back-to-back so the ~100ns PE pipeline penalty for entering
        # and leaving row-tiled mode is paid once per two chunks, not once
        # per chunk.  attn@V lags by 4 chunks (2 pairs); at a unit boundary
        # the previous unit's av tail is flushed early so its accumulators
        # are drained well before the ps_att pool wraps onto them.
        for gp in range(0, GCH, 2):
            for g in (gp, gp + 1):
                u, i = divmod(g, KCH)
                b, qq = divmod(u, QQ)
                if i == 0:
                    ps_a[u] = [
                        ps_att.tile([P, 512], f32, tag="ps_att", name=f"ps_a{u}_{h}")
                        for h in range(H_PER_CORE)
                    ]
                kbase = b * N + i * P
                qbase = b * N + qq * 512
                ps_s = ps_scores.tile([P, 1024], f32, tag="ps_s")
                for h in range(H_PER_CORE):
                    nc.tensor.matmul(
                        ps_s[:, h * 512 : (h + 1) * 512],
                        lhsT=k_sb[h * HD : (h + 1) * HD, kbase : kbase + P],
                        rhs=q_sb[h * HD : (h + 1) * HD, qbase : qbase + 512],
                        start=True,
                        stop=True,
                    )
                ex = expp.tile([P, 1024], bf16, tag="expT")
                nc.scalar.activation(out=ex, in_=ps_s, func=Exp, scale=SCALE)
                lag.append((u, i, ex))

                # unit-boundary bookkeeping rides the stream as filler tasks
                if i == 3 and u >= 1:
                    enq_task(
                        f"finA{u - 1}", g + 2, [(450, lambda v=u - 1: finish_a(v))]
                    )
                    enq_task(
                        f"finB{u - 1}", g + 4, [(220, lambda v=u - 1: finish_b(v))]
                    )
                    # deadline 3 chunks earlier than strictly needed so the
                    # last unit's stream never carries leftover stores into
                    # the serial tail
                    enq_task(
                        f"out{u - 1}",
                        g + KCH - 6,
                        [
                            (470, lambda v=u - 1, tci=tci: outproj_chunk(v, tci))
                            for tci in range(4)
                        ],
                    )

            u_now = (gp + 1) // KCH
            i_now = (gp + 1) % KCH
            while lag and (len(lag) > 4 or lag[0][0] < u_now):
                attnv(*lag.pop(0))
            pump(gp + 1, 900)

        # ---- tail ----
        # the last unit's normalize + out-projection are on the critical
        # path with nothing left to overlap, so they run fine-grained:
        # per-token-chunk multiplies, evictions split across VectorE and
        # ScalarE (idle by now), DMAs per chunk.
        for item in lag:
            attnv(*item)
        while fillerq:
            run_quantum()
        uL = NUNITS - 1
        psa0, psa1, den2 = fin.pop(uL)
        b, qq = divmod(uL, QQ)
        qbase = b * N + qq * 512
        # direct in-place normalize: Ln/Exp on the den rows at their native
        # (32-aligned) partitions, skipping the ~2.6us den2 DMA hop that is
        # pipelined away mid-kernel but serial here
        tlnT = small.tile([P, 512], f32, tag="tlnT")
        nc.scalar.activation(out=tlnT[HD : HD + 1, :], in_=psa0[HD : HD + 1, :], func=Ln)
        nc.scalar.activation(out=tlnT[32:33, :], in_=psa1[32:33, :], func=Ln)
        nc.scalar.activation(
            out=rc2full[HD : HD + 1, :], in_=tlnT[HD : HD + 1, :], func=Exp, scale=-1.0
        )
        nc.scalar.activation(
            out=rc2full[32:33, :], in_=tlnT[32:33, :], func=Exp, scale=-1.0
        )
        rrep = ps_small.tile([P, 512], f32, tag="ps_small", name="rrepL")
        nc.tensor.matmul(rrep, lhsT=selT, rhs=rc2full, start=True, stop=True)
        for tci in range(4):
            c0 = tci * P
            nc.vector.tensor_mul(
                out=anorm[0:HD, qbase + c0 : qbase + c0 + P],
                in0=psa0[0:HD, c0 : c0 + P],
                in1=rrep[0:HD, c0 : c0 + P],
            )
            for p0 in (HD, HD + 32):
                nc.vector.tensor_mul(
                    out=anorm[p0 : p0 + 32, qbase + c0 : qbase + c0 + P],
                    in0=psa1[p0 : p0 + 32, c0 : c0 + P],
                    in1=rrep[p0 : p0 + 32, c0 : c0 + P],
                )
            tok0 = qbase + c0
            ob = outst.tile([P, EMB], bf16, tag="outst")
            for e2 in range(2):
                ps = ps_small.tile([P, 512], f32, tag="ps_small")
                nc.tensor.matmul(
                    ps,
                    lhsT=anorm[:, tok0 : tok0 + P],
                    rhs=wout_sb[:, e2 * 512 : (e2 + 1) * 512],
                    start=True,
                    stop=True,
                )
                if e2 == 0:
                    nc.vector.tensor_copy(out=ob[:, 0:512], in_=ps)
                else:
                    nc.scalar.copy(out=ob[:, 512:1024], in_=ps)
            if tci < 3:
                # even the non-final stores of the last unit drain past the
                # kernel end at 11.6us/queue: split them in two, second half
                # issued from ScalarE (idle in the tail)
                nc.sync.dma_start(out=out[tok0 : tok0 + P, 0:512], in_=ob[:, 0:512])
                nc.scalar.dma_start(
                    out=out[tok0 : tok0 + P, 512:1024], in_=ob[:, 512:1024]
                )
            else:
                # the very last store gates kernel completion: split it
                # across queues (and issue engines) so its transfer tail is
                # ~3us instead of ~11.6us
                for s, eng in enumerate((nc.sync, nc.scalar, nc.gpsimd, nc.sync)):
                    eng.dma_start(
                        out=out[tok0 : tok0 + P, s * 256 : (s + 1) * 256],
                        in_=ob[:, s * 256 : (s + 1) * 256],
                    )


def _get_graph():
    if "nc" not in _CACHE:
        _CACHE["nc"] = _build_graph()
    return _CACHE["nc"]


def kernel(**inputs):
    x = np.asarray(inputs["x"], dtype=np.float32)
    W_qkv = np.asarray(inputs["W_qkv"], dtype=np.float32)
    b_qkv = np.asarray(inputs["b_qkv"], dtype=np.float32)
    W_out = np.asarray(inputs["W_out"], dtype=np.float32)
    b_out = np.asarray(inputs["b_out"], dtype=np.float32)

    nc = _get_graph()

    bf16 = ml_dtypes.bfloat16
    xT = np.ascontiguousarray(x.reshape(TOK, EMB).T).astype(bf16)
    in_maps = []
    for c in range(8):
        cols = np.concatenate(
            [
                np.arange(c * 128, (c + 1) * 128),
                np.arange(1024 + c * 128, 1024 + (c + 1) * 128),
                np.arange(2048 + c * 128, 2048 + (c + 1) * 128),
            ]
        )
        in_maps.append(
            {
                "xT": xT,
                "wqkv": np.ascontiguousarray(W_qkv[:, cols]).astype(bf16),
                "bqkv": np.ascontiguousarray(b_qkv[cols]).astype(np.float32),
                "wout": np.ascontiguousarray(
                    W_out[c * 128 : (c + 1) * 128, :]
                ).astype(bf16),
            }
        )

    from concourse.bass_utils import run_bass_kernel_spmd

    res = run_bass_kernel_spmd(nc, in_maps, core_ids=list(range(8)))
    LAST["results"] = res

    acc = np.zeros((TOK, EMB), np.float32)
    for r in res.results:
        acc += np.asarray(r["out"], dtype=np.float32)
    acc += b_out[None, :]
    # V-bias passes through softmax normalization as a constant add to the
    # attention output: attn @ (V + 1 b_v^T) / den = attn@V/den + b_v, so its
    # contribution to the output is just b_v @ W_out (the device kernel only
    # applies the K/Q biases).
    acc += b_qkv[2048:].astype(np.float32) @ W_out.astype(np.float32)
    return acc.reshape(B, N, EMB).astype(np.float32)


if __name__ == "__main__":
    rng = np.random.default_rng(0)
    inputs = {
        "x": rng.standard_normal((B, N, EMB), dtype=np.float32),
        "W_qkv": rng.standard_normal((EMB, 3072), dtype=np.float32) * EMB**-0.5,
        "b_qkv": np.zeros((3072,), np.float32),
        "W_out": rng.standard_normal((1024, EMB), dtype=np.float32) * 1024**-0.5,
        "b_out": np.zeros((1024,), np.float32),
    }
    y = kernel(**inputs)
    print("out", y.shape, y.dtype, float(np.abs(y).mean()))


# revision 62
# speedup vs baseline: 1.1894x; 1.1894x over previous
"""Multi-head attention (B=2, N=2048, EMB=1024, H=16, hd=64) on 8 TRN2 NeuronCores.

Sharding: tensor-parallel over heads. Each core owns 2 heads: it gets the
W_qkv columns (k|q|v sections) and W_out rows for those heads, computes
QKV projection + attention + its partial output projection, and the host
sums the 8 partials (the "all-reduce") and adds b_out.

Device kernel layout (per core), all matmuls bf16 with fp32 PSUM accumulation:
  - x is pre-transposed on host to xT [EMB, TOK] so the embedding dim lands on
    SBUF partitions (matmul contraction dim).
  - QKV^T is produced in [dims, tokens] layout: lhsT = W chunk, rhs = xT chunk.
    K/Q sections stay transposed ([hd, tok]) for the scores matmul; the V
    section is produced directly in [tok, hd] tiles (vaug) via plain matmuls
    with lhsT = xT chunk.
  - vaug per (b, i, h): head 0 = [v dims @ cols 0..63 | ones @ col 64],
    head 1 = [ones @ col 63 | v dims @ cols 64..127].  attn@V therefore puts
    head-1 dims straight onto PSUM partitions 64..127 (and its softmax
    denominator on row 63), so the normalized output lands on the partitions
    the out-projection needs without any partition-shift DMA.
  - scores^T chunks [k_tok 128, q 512] per head via row-packed (K=64) matmuls
    (the two heads run concurrently in disjoint PE row groups).  Entering or
    leaving row-tiled mode costs ~100ns of PE pipeline flush, so score groups
    for two consecutive k-chunks are emitted back-to-back (the penalty is
    paid once per pair).  exp on ScalarE straight out of PSUM (scale=1/8
    folded in, no max subtraction needed: scores ~ N(0,1)), bf16 expT.
  - attn@V: lhsT = vaug [k_tok 128, 128], accumulated over 16 k chunks, with
    a global 4-chunk lag behind the exp stream.  The chunk stream is a single
    software pipeline across ALL 8 (batch, q-quarter) units, so the PE never
    drains at unit boundaries.
  - normalize: psa evicted bf16 (h0 rows 0..64, h1 rows 63..127); den rows
    DMA-packed to one [2, 512] tile; one Ln + one Exp (scale=-1) per unit on
    ScalarE (same table set as the bulk exp), emitted as two separate filler
    quanta so the bulk-exp stream sees two small bumps, not one 1.4us stall;
    partition-broadcast by a single zero-padded K=128 selector-matmul (plain
    full-array op, no tile-mode switch); multiplies on VectorE into A_norm.
  - out projection: lhsT = A_norm chunk [128, 128], rhs = W_out shard
    [128, 512], PSUM -> SBUF bf16 -> DRAM partial [TOK, EMB] bf16.
"""

import os
from collections import deque

import numpy as np
import ml_dtypes

B = 2
N = 2048
EMB = 1024
TOK = B * N  # 4096
HD = 64
H_PER_CORE = 2
DIMS = 3 * H_PER_CORE * HD  # 384 qkv cols per core
ATT_LOCAL = H_PER_CORE * HD  # 128
P = 128
EC = EMB // P  # 8 embedding chunks
TCQ = TOK // 512  # 8 token chunks for the qkv projection
KCH = N // P  # 16 key chunks per batch
QQ = N // 512  # 4 query quarters per batch
NUNITS = B * QQ  # 8
GCH = NUNITS * KCH  # 128 global chunks
SCALE = HD ** -0.5

_CACHE = {}
LAST = {}


def _patch_act_tables():
    # Route Exp to natural_log_exp_and_others so the per-unit Ln calls and
    # the bulk Exp calls share one table set (no ACT_TABLE_LOAD thrash).
    # Entries keep their order, so act_func_set_id indices stay valid.
    import concourse.bacc as bacc_mod
    from concourse import mybir

    if getattr(bacc_mod, "_act_tables_patched", False):
        return
    orig = bacc_mod.get_activation_tables

    def patched(arch):
        t = orig(arch)
        E = mybir.ActivationFunctionType.Exp
        if "natural_log_exp_and_others" in t:
            for name, fns in t.items():
                if name != "natural_log_exp_and_others" and E in fns:
                    t[name] = fns - {E}
        return t

    bacc_mod.get_activation_tables = patched
    bacc_mod._act_tables_patched = True


def _build_graph():
    from concourse import bacc, mybir
    import concourse.tile as tile

    _patch_act_tables()

    nc = bacc.Bacc(
        "TRN2", target_bir_lowering=False, debug=False, num_devices=1
    )
    dt = mybir.dt
    xT = nc.dram_tensor("xT", [EMB, TOK], dt.bfloat16, kind="ExternalInput")
    wqkv = nc.dram_tensor("wqkv", [EMB, DIMS], dt.bfloat16, kind="ExternalInput")
    bqkv = nc.dram_tensor("bqkv", [DIMS], dt.float32, kind="ExternalInput")
    wout = nc.dram_tensor("wout", [ATT_LOCAL, EMB], dt.bfloat16, kind="ExternalInput")
    out = nc.dram_tensor("out", [TOK, EMB], dt.bfloat16, kind="ExternalOutput")

    with tile.TileContext(nc) as tc:
        _emit(tc, nc, xT, wqkv, bqkv, wout, out)
    nc.compile()
    return nc


def _emit(tc, nc, xT, wqkv, bqkv, wout, out):
    from contextlib import ExitStack
    import concourse.bass as bass
    from concourse import mybir
    from concourse.masks import make_identity

    dt = mybir.dt
    f32, bf16 = dt.float32, dt.bfloat16
    Exp = mybir.ActivationFunctionType.Exp
    Ln = mybir.ActivationFunctionType.Ln

    with ExitStack() as ctx:
        consts = ctx.enter_context(tc.tile_pool(name="consts", bufs=1))
        xt_pool = ctx.enter_context(tc.tile_pool(name="xt", bufs=3))
        persist = ctx.enter_context(tc.tile_pool(name="persist", bufs=1))
        expp = ctx.enter_context(tc.tile_pool(name="expp", bufs=8))
        small = ctx.enter_context(tc.tile_pool(name="small", bufs=8))
        outst = ctx.enter_context(tc.tile_pool(name="outst", bufs=4))
        ps_scores = ctx.enter_context(
            tc.tile_pool(name="ps_scores", bufs=2, space="PSUM")
        )
        ps_att = ctx.enter_context(tc.tile_pool(name="ps_att", bufs=2, space="PSUM"))
        ps_small = ctx.enter_context(
            tc.tile_pool(name="ps_small", bufs=2, space="PSUM")
        )

        # ---- early DMAs: first xT chunk + weights, so the QKV stream can
        # start the moment the warmup matmuls finish ----
        xts = {}

        def dma_xt(t, engs=None):
            # 8 whole-chunk DMAs: finer splits would halve per-queue transfer
            # time but each extra dma_start costs ~0.6us on the issuing
            # engine's queue, which becomes the bottleneck (measured: 235
            # issues put Sync at 63% busy and cost 31us of span).  At startup
            # the issues alternate Sync/Scalar so the 8-deep issue wall the
            # first QKV chain waits behind is halved.
            if engs is None:
                engs = (nc.sync,)
            xt = xt_pool.tile([P, EC, 512], bf16, tag="xt")
            for e in range(EC):
                engs[e % len(engs)].dma_start(
                    out=xt[:, e, :], in_=xT[e * P : (e + 1) * P, bass.ts(t, 512)]
                )
            xts[t] = xt

        dma_xt(0, engs=(nc.sync, nc.scalar))
        w_sb = consts.tile([P, EC, DIMS], bf16, tag="w_sb")
        for e in range(EC):
            eng = (nc.scalar, nc.sync)[e % 2]
            eng.dma_start(out=w_sb[:, e, :], in_=wqkv[e * P : (e + 1) * P, :])
        bias_sb = consts.tile([P, 3], f32, tag="bias_sb")
        nc.sync.dma_start(out=bias_sb, in_=bqkv[:].rearrange("(c p) -> p c", p=P))
        dma_xt(1)
        wout_sb = consts.tile([P, EMB], bf16, tag="wout_sb")
        nc.sync.dma_start(out=wout_sb, in_=wout[:, :])

        # warm up the exp table set as early as possible (one-time ~2.7us)
        warm = consts.tile([1, 8], f32, tag="warm")
        nc.vector.memset(warm, 1.0)
        nc.scalar.activation(out=warm, in_=warm, func=Ln, scale=1.0)
        nc.scalar.activation(out=warm, in_=warm, func=Exp, scale=1.0)

        ident = consts.tile([P, P], bf16, tag="ident")
        make_identity(nc, ident)
        # selector for the 1/den partition-broadcast: rows 0..63 get rc2[0]
        # (head 0), rows 64..127 get rc2[1] (head 1)
        # Selector for the 1/den partition-broadcast, zero-padded to K=128 so
        # the broadcast matmul is a plain full-array op (no row-tiled-mode
        # switch penalty).  Row 1 is staged on partition 0 and DMA'd into
        # place (engine ops need 32-aligned partition bases).  rc2full rows
        # 2..127 are zeroed once so the padded contraction adds exact zeros.
        sel = consts.tile([P, P], bf16, tag="sel")
        nc.vector.memset(sel, 0.0)
        nc.vector.memset(sel[0:1, 0:HD], 1.0)
        selrow = consts.tile([1, P], bf16, tag="selrow")
        nc.vector.memset(selrow, 0.0)
        nc.vector.memset(selrow[0:1, HD:P], 1.0)
        nc.sync.dma_start(out=sel[1:2, :], in_=selrow)
        rc2full = consts.tile([P, 512], bf16, tag="rc2full")
        nc.vector.memset(rc2full, 0.0)
        # tail-only selector: broadcast from rc2full rows 64 (h0 den) and
        # 32 (h1 den), both 32-aligned so they can be written in-place
        selT = consts.tile([P, P], bf16, tag="selT")
        nc.vector.memset(selT, 0.0)
        nc.vector.memset(selT[HD : HD + 1, 0:HD], 1.0)
        nc.vector.memset(selT[32:33, HD:P], 1.0)

        # [tok 128, b, chunk, h, 128]: h0 = [dims | ones@64 | junk],
        # h1 = [junk | ones@63 | dims@64..127].  Junk columns only feed
        # output rows that are never read.
        vaug = persist.tile([P, B, KCH, H_PER_CORE, P], bf16, tag="vaug")
        nc.vector.memset(vaug[:, :, :, :, :], 0.0)
        nc.vector.memset(vaug[:, :, :, 0, HD : HD + 1], 1.0)
        # h1's denominator ones-column sits at col 32 (not 63) so its PSUM
        # row lands on a 32-aligned partition: the tail can then run Ln/Exp
        # directly on the evicted tile without a partition-packing DMA
        nc.vector.memset(vaug[:, :, :, 1, 32:33], 1.0)

        k_sb = persist.tile([P, TOK], bf16, tag="k_sb")
        q_sb = persist.tile([P, TOK], bf16, tag="q_sb")
        anorm = persist.tile([P, TOK], bf16, tag="anorm")

        # ---- PE warmup: identity matmuls keep the PE busy through the HAM
        # activity window while the first DMAs land, so the QKV stream runs
        # at the warm 2.4 GHz clock from its first matmul ----
        wm_ps = ps_small.tile([P, 512], f32, tag="ps_small", name="wm_ps")
        for _ in range(84):
            nc.tensor.matmul(
                wm_ps[:, 0:128], lhsT=ident, rhs=ident, start=True, stop=True
            )

        qkv_dst = (k_sb, q_sb)

        def qkv_mms(t, d, e0, e1, ps):
            for e in range(e0, e1):
                nc.tensor.matmul(
                    ps,
                    lhsT=w_sb[:, e, d * P : (d + 1) * P],
                    rhs=xts[t][:, e, :],
                    start=(e == 0),
                    stop=(e == EC - 1),
                )

        def qkv_bias(t, d, ps):
            nc.vector.tensor_scalar_add(
                out=qkv_dst[d][:, bass.ts(t, 512)],
                in0=ps,
                scalar1=bias_sb[:, d : d + 1],
            )

        def v_mms(t, tci, ps):
            # V[tok, dim] for tokens t*512+tci*128 .. +128, accumulated into
            # column range tci*128..+128 of a shared [128, 512] PSUM bank
            # (one eviction copy per 4 sub-chunks).
            for e in range(EC):
                nc.tensor.matmul(
                    ps[:, tci * P : (tci + 1) * P],
                    lhsT=xts[t][:, e, tci * P : (tci + 1) * P],
                    rhs=w_sb[:, e, 2 * P : 3 * P],
                    start=(e == 0),
                    stop=(e == EC - 1),
                )

        def v_copy(t, ps):
            # ps cols [tci*128+h*64 .. +64] = head h dims for token sub-chunk
            # tci.  h0 -> vaug cols 0..64, h1 -> vaug cols 64..128.
            gtok = t * 512
            b, i0 = gtok // N, (gtok % N) // P
            src = ps.rearrange("p (c h d) -> p c h d", c=4, h=2)
            nc.vector.tensor_copy(
                out=vaug[:, b, i0 : i0 + 4, 0, 0:HD], in_=src[:, :, 0, :]
            )
            nc.vector.tensor_copy(
                out=vaug[:, b, i0 : i0 + 4, 1, HD:P], in_=src[:, :, 1, :]
            )

        def v_copy2(t, tci0, ps):
            # same as v_copy but for the tci pair [tci0, tci0+1]
            gtok = t * 512
            b, i0 = gtok // N, (gtok % N) // P
            src = ps[:, tci0 * P : (tci0 + 2) * P].rearrange(
                "p (c h d) -> p c h d", c=2, h=2
            )
            nc.vector.tensor_copy(
                out=vaug[:, b, i0 + tci0 : i0 + tci0 + 2, 0, 0:HD],
                in_=src[:, :, 0, :],
            )
            nc.vector.tensor_copy(
                out=vaug[:, b, i0 + tci0 : i0 + tci0 + 2, 1, HD:P],
                in_=src[:, :, 1, :],
            )

        # ---- filler task queue: (taskid, pe_cost_ns, fn) ----
        fillerq = deque()
        task_deadline = {}

        def enq_task(taskid, deadline, quanta):
            task_deadline[taskid] = deadline
            for cost, fn in quanta:
                fillerq.append((taskid, cost, fn))

        def run_quantum():
            _, _, fn = fillerq.popleft()
            fn()

        def pump(g, budget):
            # run everything whose deadline is upon us, then fill the budget
            while fillerq and task_deadline[fillerq[0][0]] <= g + 1:
                run_quantum()
            while fillerq and budget > 0:
                tid, cost, _ = fillerq[0]
                run_quantum()
                budget -= cost

        # Each quantum fully contains the lifetime of any ps_small tile it
        # allocates (alloc + all writes + final reader), so the pool's
        # round-robin reuse can never interleave with a half-written bank.
        def make_qkv_task(t):
            quanta = []
            for d in range(2):

                def q(d=d):
                    ps = ps_small.tile(
                        [P, 512], f32, tag="ps_small", name=f"qkv{t}_{d}"
                    )
                    qkv_mms(t, d, 0, EC, ps)
                    qkv_bias(t, d, ps)

                quanta.append((1760, q))
            return quanta

        def make_v_task(t):
            # two ~0.9us quanta (tci pairs) instead of one 1.8us block, so a
            # popped v quantum delays the next scores batch half as long.
            # Each quantum owns its PSUM bank for its full lifetime.
            def q(tci0):
                ps = ps_small.tile(
                    [P, 512], f32, tag="ps_small", name=f"v{t}_{tci0}"
                )
                for tci in (tci0, tci0 + 1):
                    v_mms(t, tci, ps)
                v_copy2(t, tci0, ps)

            return [(900, lambda: q(0)), (900, lambda: q(2))]

        # t=0 runs inline before the stream; t=1..7 go through the queue.
        # Deadlines (global chunk index): batch-0 k-side needs qkv(t) by
        # chunk 4t; batch-1 needs qkv(4+j) by chunk 64+4j.  The xT DMA for
        # t+1 is enqueued ahead of qkv(t)'s matmuls so the transfer overlaps
        # a full task's worth of compute.
        def qkv_ddl(t):
            return 4 * t if t < 4 else 64 + 4 * (t - 4)

        # v0 is only needed by the first attn@V (chunk 4): run it as the
        # first filler quanta instead of ahead of the first scores pair
        enq_task("v0", 2, make_v_task(0))
        for t in range(1, TCQ):
            if t + 1 < TCQ:
                enq_task(
                    f"dma{t + 1}",
                    max(qkv_ddl(t + 1) - 4, 0),
                    [(0, lambda t2=t + 1: dma_xt(t2))],
                )
            enq_task(f"qkv{t}", qkv_ddl(t), make_qkv_task(t))
            enq_task(f"v{t}", qkv_ddl(t) + 2, make_v_task(t))

        # ---- t=0 inline (the stream's first chunks need it) ----
        # The K and Q accumulation chains interleave per e-chunk: the first
        # QKV is gated on the staggered xt0 DMA arrivals, so this way both
        # chains finish ~one chain-length sooner after the last arrival.
        ps0 = ps_small.tile([P, 512], f32, tag="ps_small", name="qkv0_0")
        ps1 = ps_small.tile([P, 512], f32, tag="ps_small", name="qkv0_1")
        for e in range(EC):
            for d, ps in ((0, ps0), (1, ps1)):
                nc.tensor.matmul(
                    ps,
                    lhsT=w_sb[:, e, d * P : (d + 1) * P],
                    rhs=xts[0][:, e, :],
                    start=(e == 0),
                    stop=(e == EC - 1),
                )
        qkv_bias(0, 0, ps0)
        qkv_bias(0, 1, ps1)

        # ---- per-unit finish machinery ----
        fin = {}

        def drain_unit(u, ps_a):
            # evict accumulators to SBUF (bf16): h0 rows 0..64 (den at 64),
            # h1 rows 63..127 (den at 63)
            psa0 = small.tile([P, 512], bf16, tag="psa_sb")
            nc.vector.tensor_copy(out=psa0[0 : HD + 1, :], in_=ps_a[0][0 : HD + 1, :])
            psa1 = small.tile([P, 512], bf16, tag="psa_sb")
            # non-zero partition bases are limited to 32-partition accesses,
            # and cost scales with the free dim only, so copy the full tile
            # (rows 0..62 are junk, den at 63, dims at 64..127).  For the
            # last unit the copy runs on ScalarE (idle in the tail) so both
            # evictions overlap instead of serializing on VectorE.
            if u == NUNITS - 1:
                nc.scalar.copy(out=psa1, in_=ps_a[1][:, :])
            else:
                nc.vector.tensor_copy(out=psa1, in_=ps_a[1][:, :])
            den2 = small.tile([2, 512], bf16, tag="den2")
            nc.sync.dma_start(out=den2[0:1, :], in_=psa0[HD : HD + 1, :])
            nc.sync.dma_start(out=den2[1:2, :], in_=psa1[32:33, :])
            fin[u] = (psa0, psa1, den2)

        # 1/den = exp(-ln(den)) on ScalarE, one [2, 512] call per func;
        # partition-broadcast via a single K=128 zero-padded selector
        # matmul; then normalize multiplies on VectorE.  Split into two
        # quanta so the ScalarE insertions are two ~0.7us bumps instead of
        # one 1.4us block that stalls the bulk-exp stream.
        def finish_a(u):
            _, _, den2 = fin[u]
            tln = small.tile([2, 512], f32, tag="tln")
            nc.scalar.activation(out=tln, in_=den2, func=Ln)
            fin[u] = fin[u][:2] + (tln,)

        def finish_b(u):
            psa0, psa1, tln = fin.pop(u)
            b, qq = divmod(u, QQ)
            qbase = b * N + qq * 512
            nc.scalar.activation(out=rc2full[0:2, :], in_=tln, func=Exp, scale=-1.0)
            rrep = ps_small.tile([P, 512], f32, tag="ps_small", name=f"rrep{u}")
            nc.tensor.matmul(rrep, lhsT=sel, rhs=rc2full, start=True, stop=True)
            nc.vector.tensor_mul(
                out=anorm[0:HD, qbase : qbase + 512],
                in0=psa0[0:HD, :],
                in1=rrep[0:HD, :],
            )
            # >32-partition accesses must start at partition 0, so the h1
            # multiply goes in two 32-partition pieces
            for p0 in (HD, HD + 32):
                nc.vector.tensor_mul(
                    out=anorm[p0 : p0 + 32, qbase : qbase + 512],
                    in0=psa1[p0 : p0 + 32, :],
                    in1=rrep[p0 : p0 + 32, :],
                )

        def finish_unit(u):
            finish_a(u)
            finish_b(u)

        def outproj_chunk(u, tci):
            b, qq = divmod(u, QQ)
            tok0 = b * N + qq * 512 + tci * P
            ob = outst.tile([P, EMB], bf16, tag="outst")
            for e2 in range(2):
                ps = ps_small.tile([P, 512], f32, tag="ps_small")
                nc.tensor.matmul(
                    ps,
                    lhsT=anorm[:, tok0 : tok0 + P],
                    rhs=wout_sb[:, e2 * 512 : (e2 + 1) * 512],
                    start=True,
                    stop=True,
                )
                nc.vector.tensor_copy(out=ob[:, e2 * 512 : (e2 + 1) * 512], in_=ps)
            nc.sync.dma_start(out=out[tok0 : tok0 + P, :], in_=ob)

        # ---- the global chunk stream ----
        # One software pipeline over all 128 (unit, k-chunk) pairs: attn@V
        # lags the exp stream by 2 chunks and flows straight across unit
        # boundaries, so the PE never waits for an exp at a boundary.
        ps_a = {}
        lag = []

        def attnv(u, i, ex):
            b = u // QQ
            for h in range(H_PER_CORE):
                nc.tensor.matmul(
                    ps_a[u][h],
                    lhsT=vaug[:, b, i, h, :],
                    rhs=ex[:, h * 512 : (h + 1) * 512],
                    start=(i == 0),
                    stop=(i == KCH - 1),
                )
            if i == KCH - 1:
                drain_unit(u, ps_a.pop(u))

        # Chunks are processed in PAIRS: the two row-packed score groups are
        # emitted back-to-back so the ~100ns PE pipeline penalty for entering
        # and leaving row-tiled mode is paid once per two chunks, not once
        # per chunk.  attn@V lags by 4 chunks (2 pairs); at a unit boundary
        # the previous unit's av tail is flushed early so its accumulators
        # are drained well before the ps_att pool wraps onto them.
        for gp in range(0, GCH, 2):
            for g in (gp, gp + 1):
                u, i = divmod(g, KCH)
                b, qq = divmod(u, QQ)
                if i == 0:
                    ps_a[u] = [
                        ps_att.tile([P, 512], f32, tag="ps_att", name=f"ps_a{u}_{h}")
                        for h in range(H_PER_CORE)
                    ]
                kbase = b * N + i * P
                qbase = b * N + qq * 512
                ps_s = ps_scores.tile([P, 1024], f32, tag="ps_s")
                for h in range(H_PER_CORE):
                    nc.tensor.matmul(
                        ps_s[:, h * 512 : (h + 1) * 512],
                        lhsT=k_sb[h * HD : (h + 1) * HD, kbase : kbase + P],
                        rhs=q_sb[h * HD : (h + 1) * HD, qbase : qbase + 512],
                        start=True,
                        stop=True,
                    )
                ex = expp.tile([P, 1024], bf16, tag="expT")
                nc.scalar.activation(out=ex, in_=ps_s, func=Exp, scale=SCALE)
                lag.append((u, i, ex))

                # unit-boundary bookkeeping rides the stream as filler tasks
                if i == 3 and u >= 1:
                    enq_task(
                        f"finA{u - 1}", g + 2, [(450, lambda v=u - 1: finish_a(v))]
                    )
                    enq_task(
                        f"finB{u - 1}", g + 4, [(220, lambda v=u - 1: finish_b(v))]
                    )
                    # deadline 3 chunks earlier than strictly needed so the
                    # last unit's stream never carries leftover stores into
                    # the serial tail
                    enq_task(
                        f"out{u - 1}",
                        g + KCH - 6,
                        [
                            (470, lambda v=u - 1, tci=tci: outproj_chunk(v, tci))
                            for tci in range(4)
                        ],
                    )

            u_now = (gp + 1) // KCH
            i_now = (gp + 1) % KCH
            while lag and (len(lag) > 4 or lag[0][0] < u_now):
                attnv(*lag.pop(0))
            pump(gp + 1, 900)

        # ---- tail ----
        # the last unit's normalize + out-projection are on the critical
        # path with nothing left to overlap, so they run fine-grained:
        # per-token-chunk multiplies, evictions split across VectorE and
        # ScalarE (idle by now), DMAs per chunk.
        for item in lag:
            attnv(*item)
        while fillerq:
            run_quantum()
        uL = NUNITS - 1
        psa0, psa1, den2 = fin.pop(uL)
        b, qq = divmod(uL, QQ)
        qbase = b * N + qq * 512
        # direct in-place normalize: Ln/Exp on the den rows at their native
        # (32-aligned) partitions, skipping the ~2.6us den2 DMA hop that is
        # pipelined away mid-kernel but serial here
        tlnT = small.tile([P, 512], f32, tag="tlnT")
        nc.scalar.activation(out=tlnT[HD : HD + 1, :], in_=psa0[HD : HD + 1, :], func=Ln)
        nc.scalar.activation(out=tlnT[32:33, :], in_=psa1[32:33, :], func=Ln)
        nc.scalar.activation(
            out=rc2full[HD : HD + 1, :], in_=tlnT[HD : HD + 1, :], func=Exp, scale=-1.0
        )
        nc.scalar.activation(
            out=rc2full[32:33, :], in_=tlnT[32:33, :], func=Exp, scale=-1.0
        )
        rrep = ps_small.tile([P, 512], f32, tag="ps_small", name="rrepL")
        nc.tensor.matmul(rrep, lhsT=selT, rhs=rc2full, start=True, stop=True)
        for tci in range(4):
            c0 = tci * P
            nc.vector.tensor_mul(
                out=anorm[0:HD, qbase + c0 : qbase + c0 + P],
                in0=psa0[0:HD, c0 : c0 + P],
                in1=rrep[0:HD, c0 : c0 + P],
            )
            for p0 in (HD, HD + 32):
                nc.vector.tensor_mul(
                    out=anorm[p0 : p0 + 32, qbase + c0 : qbase + c0 + P],
                    in0=psa1[p0 : p0 + 32, c0 : c0 + P],
                    in1=rrep[p0 : p0 + 32, c0 : c0 + P],
                )
            tok0 = qbase + c0
            ob = outst.tile([P, EMB], bf16, tag="outst")
            # the tail pipeline is DVE-gated (it owns the 3 normalize muls
            # per chunk): both output evictions go to the tail-idle ScalarE,
            # and each half-store issues right after its own eviction so the
            # transfers start as early as possible
            for e2 in range(2):
                ps = ps_small.tile([P, 512], f32, tag="ps_small")
                nc.tensor.matmul(
                    ps,
                    lhsT=anorm[:, tok0 : tok0 + P],
                    rhs=wout_sb[:, e2 * 512 : (e2 + 1) * 512],
                    start=True,
                    stop=True,
                )
                if e2 == 0:
                    nc.vector.tensor_copy(out=ob[:, 0:512], in_=ps)
                    nc.sync.dma_start(
                        out=out[tok0 : tok0 + P, 0:512], in_=ob[:, 0:512]
                    )
                else:
                    nc.scalar.copy(out=ob[:, 512:1024], in_=ps)
                    if tci < 3:
                        nc.scalar.dma_start(
                            out=out[tok0 : tok0 + P, 512:1024], in_=ob[:, 512:1024]
                        )
                    else:
                        # the very last store gates kernel completion: split
                        # it across queues so its transfer tail is short
                        nc.scalar.dma_start(
                            out=out[tok0 : tok0 + P, 512:768], in_=ob[:, 512:768]
                        )
                        nc.gpsimd.dma_start(
                            out=out[tok0 : tok0 + P, 768:1024], in_=ob[:, 768:1024]
                        )


def _get_graph():
    if "nc" not in _CACHE:
        _CACHE["nc"] = _build_graph()
    return _CACHE["nc"]


def kernel(**inputs):
    x = np.asarray(inputs["x"], dtype=np.float32)
    W_qkv = np.asarray(inputs["W_qkv"], dtype=np.float32)
    b_qkv = np.asarray(inputs["b_qkv"], dtype=np.float32)
    W_out = np.asarray(inputs["W_out"], dtype=np.float32)
    b_out = np.asarray(inputs["b_out"], dtype=np.float32)

    nc = _get_graph()

    bf16 = ml_dtypes.bfloat16
    xT = np.ascontiguousarray(x.reshape(TOK, EMB).T).astype(bf16)
    in_maps = []
    for c in range(8):
        cols = np.concatenate(
            [
                np.arange(c * 128, (c + 1) * 128),
                np.arange(1024 + c * 128, 1024 + (c + 1) * 128),
                np.arange(2048 + c * 128, 2048 + (c + 1) * 128),
            ]
        )
        in_maps.append(
            {
                "xT": xT,
                "wqkv": np.ascontiguousarray(W_qkv[:, cols]).astype(bf16),
                "bqkv": np.ascontiguousarray(b_qkv[cols]).astype(np.float32),
                "wout": np.ascontiguousarray(
                    W_out[c * 128 : (c + 1) * 128, :]
                ).astype(bf16),
            }
        )

    from concourse.bass_utils import run_bass_kernel_spmd

    res = run_bass_kernel_spmd(nc, in_maps, core_ids=list(range(8)))
    LAST["results"] = res

    acc = np.zeros((TOK, EMB), np.float32)
    for r in res.results:
        acc += np.asarray(r["out"], dtype=np.float32)
    acc += b_out[None, :]
    # V-bias passes through softmax normalization as a constant add to the
    # attention output: attn @ (V + 1 b_v^T) / den = attn@V/den + b_v, so its
    # contribution to the output is just b_v @ W_out (the device kernel only
    # applies the K/Q biases).
    acc += b_qkv[2048:].astype(np.float32) @ W_out.astype(np.float32)
    return acc.reshape(B, N, EMB).astype(np.float32)


if __name__ == "__main__":
    rng = np.random.default_rng(0)
    inputs = {
        "x": rng.standard_normal((B, N, EMB), dtype=np.float32),
        "W_qkv": rng.standard_normal((EMB, 3072), dtype=np.float32) * EMB**-0.5,
        "b_qkv": np.zeros((3072,), np.float32),
        "W_out": rng.standard_normal((1024, EMB), dtype=np.float32) * 1024**-0.5,
        "b_out": np.zeros((1024,), np.float32),
    }
    y = kernel(**inputs)
    print("out", y.shape, y.dtype, float(np.abs(y).mean()))
